# revision 11
# baseline (speedup 1.0000x reference)
"""Trainium2 Bass kernel for nn_Lookback: causal running-mean over T.

out[b, t, c] = (1/(t+1)) * sum_{s<=t} x[b, s, c],  x: [8, 4096, 1024] fp32.

Sharding: data-parallel over batch B — core b handles x[b] ([4096, 1024]).

The 2e-2 rel-err budget allows bf16 I/O: the host casts x to bf16, the
kernel streams bf16 and writes bf16 out, halving HBM traffic vs the f32
baseline.  DRAM buffers are laid out partition-major ([P, NT*C]) so every
DMA is per-partition contiguous (128 large descriptors / transfer).
Loads issue on the SP HWDGE ring, stores on the GPSIMD SWDGE ring, so
neither blocks the other (per-ring FIFO) and the ACT engine is free for
evictions.

Per-core algorithm (T tiled into 32 blocks of P=128 rows, 4 segments of
8 tiles, pipelined so segment s+1's load overlaps segment s's phase B):
  Phase A: tile column-sums  totals[j, c] = sum_p x_j[p, c]
           as a PSUM accumulation of matmuls with indicator weights E_j.
  Phase B: out_k = tril128 @ x_k + G_k @ totals, processed in PAIRS:
           the two carry matmuls of a pair run concurrently on PE row
           strips 0 / 32 (K=32 row tiling; G_k staged at base partition
           32*(k%2), totals replicated to [64, C] bf16).
  Scale by d[t] = 1/(t+1) during PSUM->SBUF eviction, split per tile
  across DVE (cols 0:512) and ACT (cols 512:1024) so eviction latency
  (~0.64us) stays under the PE work between PSUM-buffer reuses.
  PE warm-up runs on a memset tile so it needs no DMA and the HAM clock
  gate reaches 8/8 before the first real matmul.

The whole matmul path is bf16 (f32 PSUM accumulation); the f32 running
totals live in SBUF, updated per segment on the DVE.
"""

import sys

import numpy as np

sys.path.insert(0, "/opt/trn_rl_repo")

import ml_dtypes

import concourse.bass as bass
import concourse.mybir as mybir
import concourse.tile as tile
from concourse import bacc
from concourse.bass_utils import run_bass_kernel_spmd

B, T, C = 8, 4096, 1024
P = 128
NT = T // P          # 32 row tiles per core
NSEG = 4
SEG = NT // NSEG     # 8 tiles per segment
CH = 512             # PSUM bank chunk (fp32)
NCH = C // CH
DB = 4               # tiles per store batch (1 MiB in bf16)
F32 = mybir.dt.float32
BF16 = mybir.dt.bfloat16

_cache = {}


def _consts():
    """Host-precomputed weight matrices (shared by all cores)."""
    # trilT[q, p] = [q <= p]  (lhsT of the lower-triangular ones matrix)
    tril_t = np.tril(np.ones((P, P), np.float32)).T.copy()
    # E_all[:, k*NT:(k+1)*NT] = E_k with E_k[p, m] = [m == k] (global row)
    e_all = np.zeros((P, NT * NT), np.float32)
    for k in range(NT):
        e_all[:, k * NT + k] = 1.0
    wconst = np.concatenate([tril_t, e_all], axis=1)  # [P, P + NT*NT]
    # G2: carry weights for tile k at base partition 32*(k%2)
    g2 = np.zeros((2 * NT, NT * P), np.float32)
    for k in range(NT):
        off = NT * (k % 2)
        g2[off:off + k, k * P:(k + 1) * P] = 1.0
    # recip[p, k] = 1 / (128*k + p + 1)
    t_idx = np.arange(T, dtype=np.float64).reshape(NT, P).T  # [P, NT]
    recip = (1.0 / (t_idx + 1.0)).astype(np.float32)
    bf = ml_dtypes.bfloat16
    return wconst.astype(bf), g2.astype(bf), recip


def _build():
    nc = bacc.Bacc("TRN2", target_bir_lowering=False, debug=False, num_devices=B)
    # partition-major layouts: element (k, p, c) lives at [p, k*C + c]
    x_d = nc.dram_tensor("x", [P, NT * C], BF16, kind="ExternalInput").ap()
    w_d = nc.dram_tensor("wconst", [P, P + NT * NT], BF16,
                         kind="ExternalInput").ap()
    g_d = nc.dram_tensor("g2", [2 * NT, NT * P], BF16, kind="ExternalInput").ap()
    r_d = nc.dram_tensor("recip", [P, NT], F32, kind="ExternalInput").ap()
    out_d = nc.dram_tensor("out", [P, NT * C], BF16, kind="ExternalOutput").ap()

    with tile.TileContext(nc) as tc:
        with (
            tc.tile_pool(name="const", bufs=1) as cp,
            tc.tile_pool(name="xres", bufs=1) as xp,
            tc.tile_pool(name="tot", bufs=1) as tp,
            tc.tile_pool(name="ev", bufs=1) as ep,
            tc.tile_pool(name="ps", bufs=3, space=bass.MemorySpace.PSUM) as psp,
            tc.tile_pool(name="pt", bufs=1, space=bass.MemorySpace.PSUM) as ptp,
        ):
            w_s = cp.tile([P, P + NT * NT], BF16)
            g_s = cp.tile([2 * NT, NT * P], BF16)
            r_s = cp.tile([P, NT], F32)
            warm = cp.tile([P, P + CH], BF16)
            nc.sync.dma_start(w_s[:], w_d)
            nc.scalar.dma_start(g_s[:], g_d)
            nc.scalar.dma_start(r_s[:], r_d)
            tril_s = w_s[:, 0:P]
            e_s = w_s[:, P:P + NT * NT]

            xr = xp.tile([P, NT * C], BF16)           # resident input
            # out staging: rotating bf16 buffers of DB tiles each
            ostage = [
                ep.tile([P, DB * C], BF16, tag=f"o{i}", name=f"o{i}")
                for i in range(3)
            ]
            totb_list = []

            # PE warm-up on a memset tile — no DMA dependency, so it runs
            # during the preamble and the HAM clock gate is already 8/8
            # (2.4 GHz) when the first real matmul issues.
            nc.vector.memset(warm[:], 0)
            dmy = psp.tile([P, CH], F32, tag="ps")
            for _ in range(6):
                nc.tensor.matmul(dmy[:], warm[:, 0:P], warm[:, P:P + CH],
                                 start=True, stop=True)

            def load_batch(k0, n):
                sl = slice(k0 * C, (k0 + n) * C)
                nc.sync.dma_start(xr[:, sl], x_d[:, sl])

            # graduated first loads so phase A can start ~4us earlier
            for kk, n in ((0, 1), (1, 1), (2, 2), (4, 4)):
                load_batch(kk, n)

            for s in range(NSEG):
                k0, k1 = s * SEG, (s + 1) * SEG
                pt = ptp.tile([NT, C], F32)
                # ---- phase A (tile 31's column sum is never used) -----
                ka_end = k1 - 1 if s == NSEG - 1 else k1
                for k in range(k0, ka_end):
                    xs = xr[:, k * C:(k + 1) * C]
                    for h in range(NCH):
                        sl = slice(h * CH, (h + 1) * CH)
                        nc.tensor.matmul(
                            pt[:, sl],
                            e_s[:, k * NT:(k + 1) * NT],
                            xs[:, sl],
                            start=(k == k0),
                            stop=(k == ka_end - 1),
                        )
                # prefetch next segment's input
                if s + 1 < NSEG:
                    for kk in range((s + 1) * SEG, (s + 2) * SEG, DB):
                        load_batch(kk, DB)
                # running totals, accumulated directly in bf16 at base
                # partitions 0 and 32 (the two carry row strips).  The
                # strip-32 add goes first: the first pair's second tile
                # (strip 32) is the earliest in-segment consumer, ready
                # after ~1.2us instead of a 2.4us serialized chain.
                totb_s = tp.tile([2 * NT, C], BF16, tag=f"tb{s}", name=f"tb{s}")
                if s == 0:
                    nc.vector.tensor_copy(totb_s[NT:2 * NT, :], pt[:])
                    nc.vector.tensor_copy(totb_s[0:NT, :], pt[:])
                else:
                    prev = totb_list[s - 1]
                    nc.vector.tensor_add(
                        totb_s[NT:2 * NT, :], prev[NT:2 * NT, :], pt[:])
                    nc.vector.tensor_add(
                        totb_s[0:NT, :], prev[0:NT, :], pt[:])
                totb_list.append(totb_s)

                # ---- phase B: pairs (a, b); carries of a pair run
                # concurrently on PE row strips 0 / 32 ------------------
                stage = [None] * SEG

                def tril_mm(k):
                    xs = xr[:, k * C:(k + 1) * C]
                    ps = psp.tile([P, C], F32)
                    stage[k - k0] = ps
                    for h in range(NCH):
                        sl = slice(h * CH, (h + 1) * CH)
                        nc.tensor.matmul(
                            ps[:, sl], tril_s[:], xs[:, sl],
                            start=True, stop=(k == 0),
                        )

                def carry_wave(a, b):
                    for h in range(NCH):
                        sl = slice(h * CH, (h + 1) * CH)
                        for k in (a, b):
                            if k == 0:
                                continue
                            # first tile of a segment only needs rows
                            # j < k0, final in the previous totals
                            tb = (totb_list[s - 1]
                                  if (k == k0 and s > 0) else totb_s)
                            off = NT * (k % 2)
                            nc.tensor.matmul(
                                stage[k - k0][:, sl],
                                g_s[off:off + NT, k * P:(k + 1) * P],
                                tb[off:off + NT, sl],
                                start=False, stop=True,
                            )

                def evict(k):
                    ps = stage[k - k0]
                    ob = k // DB
                    o = ostage[ob % len(ostage)]
                    osl = o[:, (k % DB) * C:((k % DB) + 1) * C]
                    scale = r_s[:, k:k + 1]
                    # DVE also runs the totals adds, so it gets the
                    # smaller share: DVE 448 cols / ACT 576 cols
                    nc.vector.tensor_scalar_mul(
                        osl[:, 0:448], ps[:, 0:448], scale)
                    nc.scalar.activation(
                        osl[:, 448:C], ps[:, 448:C],
                        mybir.ActivationFunctionType.Copy, scale=scale,
                    )
                    # stores: 1 MiB batches; the final batch ships as
                    # single tiles so the tail drains fast
                    last = (s == NSEG - 1) and (k >= NT - DB)
                    if last:
                        h0 = k % DB
                        sl = slice(k * C, (k + 1) * C)
                        nc.gpsimd.dma_start(
                            out_d[:, sl], o[:, h0 * C:(h0 + 1) * C])
                    elif k % DB == DB - 1:
                        sl = slice((k - DB + 1) * C, (k + 1) * C)
                        nc.gpsimd.dma_start(out_d[:, sl], o[:])

                for a in range(k0, k1, 2):
                    b = a + 1
                    tril_mm(a)
                    tril_mm(b)
                    carry_wave(a, b)
                    evict(a)
                    evict(b)

    nc.compile()
    return nc


def _run(x, trace=False):
    assert x.shape == (B, T, C)
    # partition-major bf16 layout: [B, P, NT*C]
    xb = (
        np.ascontiguousarray(x)
        .astype(ml_dtypes.bfloat16)
        .reshape(B, NT, P, C)
        .transpose(0, 2, 1, 3)
        .reshape(B, P, NT * C)
        .copy()
    )
    if "nc" not in _cache:
        _cache["nc"] = _build()
        _cache["consts"] = _consts()
    nc = _cache["nc"]
    wconst, g2, recip = _cache["consts"]
    in_maps = [
        {"x": xb[b], "wconst": wconst, "g2": g2, "recip": recip}
        for b in range(B)
    ]
    res = run_bass_kernel_spmd(nc, in_maps, core_ids=list(range(B)), trace=trace)
    out = np.stack([
        res.results[b]["out"]
        .astype(np.float32)
        .reshape(P, NT, C)
        .transpose(1, 0, 2)
        .reshape(T, C)
        for b in range(B)
    ])
    return out, res


def kernel(x):
    out, _ = _run(x, trace=False)
    return out


# revision 15
# speedup vs baseline: 1.0254x; 1.0254x over previous
"""Trainium2 Bass kernel for nn_Lookback: causal running-mean over T.

out[b, t, c] = (1/(t+1)) * sum_{s<=t} x[b, s, c],  x: [8, 4096, 1024] fp32.

Sharding: data-parallel over batch B — core b handles x[b] ([4096, 1024]).

The 2e-2 rel-err budget allows bf16 I/O: the host casts x to bf16, the
kernel streams bf16 and writes bf16 out, halving HBM traffic vs the f32
baseline.  DRAM buffers are laid out partition-major ([P, NT*C]) so every
DMA is per-partition contiguous (128 large descriptors / transfer).
Loads issue on the SP HWDGE ring, stores on the GPSIMD SWDGE ring, so
neither blocks the other (per-ring FIFO) and the ACT engine is free for
evictions.

Per-core algorithm (T tiled into 32 blocks of P=128 rows, 4 segments of
8 tiles, pipelined so segment s+1's load overlaps segment s's phase B):
  Phase A: per-segment tile column-sums into PSUM rows 0..7 via indicator
           weights E_j (pt[j] = colsum of the segment's j-th tile).
  TOT:     pt is copied (bf16, chunked on DVE) into strip s (rows
           8s..8s+8) of a persistent [32, C] totals tile.  Cross-segment
           accumulation is folded into the carry weights instead of a
           serialized DVE add chain: for tile k (segment sk, local lk)
           the carry matmul contracts over K = 8*sk + lk rows of TOT,
           with weight 1 on full previous strips and [j < lk] on the
           current strip.
  Phase B: out_k = tril128 @ x_k  (+ carry matmul into the same PSUM).
  Scale by d[t] = 1/(t+1) during PSUM->SBUF eviction, split per tile
  across DVE (cols 0:448) and ACT (cols 448:1024), into bf16 staging
  tiles DMA'd out 4 tiles (1 MiB) at a time (single tiles for the last
  batch so the tail drains fast).
  PE warm-up runs on a memset tile so it needs no DMA and the HAM clock
  gate reaches 8/8 before the first real matmul.

The whole matmul path is bf16 (f32 PSUM accumulation).
"""

import sys

import numpy as np

sys.path.insert(0, "/opt/trn_rl_repo")

import ml_dtypes

import concourse.bass as bass
import concourse.mybir as mybir
import concourse.tile as tile
from concourse import bacc
from concourse.bass_utils import run_bass_kernel_spmd

B, T, C = 8, 4096, 1024
P = 128
NT = T // P          # 32 row tiles per core
NSEG = 4
SEG = NT // NSEG     # 8 tiles per segment
CH = 512             # PSUM bank chunk (fp32)
NCH = C // CH
DB = 4               # tiles per store batch (1 MiB in bf16)
DVE_COLS = 448       # eviction split: DVE 448 / ACT 576
F32 = mybir.dt.float32
BF16 = mybir.dt.bfloat16

_cache = {}


def _consts():
    """Host-precomputed weight matrices (shared by all cores)."""
    # trilT[q, p] = [q <= p]  (lhsT of the lower-triangular ones matrix)
    tril_t = np.tril(np.ones((P, P), np.float32)).T.copy()
    # E8[:, k*SEG + j] = [j == k % SEG]: phase A maps tile k to pt row k%SEG
    e8 = np.zeros((P, NT * SEG), np.float32)
    for k in range(NT):
        e8[:, k * SEG + (k % SEG)] = 1.0
    wconst = np.concatenate([tril_t, e8], axis=1)  # [P, P + NT*SEG]
    # carry weights: strips of SEG rows at base partition 32*s (DVE
    # writes need 32-aligned partition offsets); contraction for tile k
    # covers K_k = 32*sk + lk rows: full previous strips are 1 (their
    # unused rows multiply memset zeros), current strip row j is [j < lk]
    gb = np.zeros((P, NT * P), np.float32)
    for k in range(NT):
        sk, lk = divmod(k, SEG)
        for sp in range(sk):
            gb[32 * sp:32 * sp + SEG, k * P:(k + 1) * P] = 1.0
        gb[32 * sk:32 * sk + lk, k * P:(k + 1) * P] = 1.0
    # recip[p, k] = 1 / (128*k + p + 1)
    t_idx = np.arange(T, dtype=np.float64).reshape(NT, P).T  # [P, NT]
    recip = (1.0 / (t_idx + 1.0)).astype(np.float32)
    bf = ml_dtypes.bfloat16
    return wconst.astype(bf), gb.astype(bf), recip


def _build():
    nc = bacc.Bacc("TRN2", target_bir_lowering=False, debug=False, num_devices=B)
    # partition-major layouts: element (k, p, c) lives at [p, k*C + c]
    x_d = nc.dram_tensor("x", [P, NT * C], BF16, kind="ExternalInput").ap()
    w_d = nc.dram_tensor("wconst", [P, P + NT * SEG], BF16,
                         kind="ExternalInput").ap()
    g_d = nc.dram_tensor("gb", [P, NT * P], BF16, kind="ExternalInput").ap()
    r_d = nc.dram_tensor("recip", [P, NT], F32, kind="ExternalInput").ap()
    out_d = nc.dram_tensor("out", [P, NT * C], BF16, kind="ExternalOutput").ap()

    with tile.TileContext(nc) as tc:
        with (
            tc.tile_pool(name="const", bufs=1) as cp,
            tc.tile_pool(name="xres", bufs=1) as xp,
            tc.tile_pool(name="tot", bufs=1) as tp,
            tc.tile_pool(name="ev", bufs=1) as ep,
            tc.tile_pool(name="ps", bufs=3, space=bass.MemorySpace.PSUM) as psp,
            tc.tile_pool(name="pt", bufs=1, space=bass.MemorySpace.PSUM) as ptp,
        ):
            w_s = cp.tile([P, P + NT * SEG], BF16)
            g_s = cp.tile([P, NT * P], BF16)
            r_s = cp.tile([P, NT], F32)
            warm = cp.tile([P, P + CH], BF16)
            nc.sync.dma_start(w_s[:], w_d)
            nc.scalar.dma_start(g_s[:], g_d)
            nc.scalar.dma_start(r_s[:], r_d)
            tril_s = w_s[:, 0:P]
            e_s = w_s[:, P:P + NT * SEG]

            xr = xp.tile([P, NT * C], BF16)           # resident input
            tot = tp.tile([P, C], BF16)               # per-segment strips
            # out staging: rotating bf16 buffers of DB tiles each
            ostage = [
                ep.tile([P, DB * C], BF16, tag=f"o{i}", name=f"o{i}")
                for i in range(3)
            ]

            # PE warm-up on a memset tile — no DMA dependency, so it runs
            # during the preamble and the HAM clock gate is already 8/8
            # (2.4 GHz) when the first real matmul issues.  12 matmuls
            # bridge until the first load lands (~12.5us).
            nc.vector.memset(warm[:], 0)
            nc.vector.memset(tot[:], 0)
            dmy = psp.tile([P, CH], F32, tag="ps")
            for _ in range(12):
                nc.tensor.matmul(dmy[:], warm[:, 0:P], warm[:, P:P + CH],
                                 start=True, stop=True)

            def load_batch(k0, n):
                sl = slice(k0 * C, (k0 + n) * C)
                nc.sync.dma_start(xr[:, sl], x_d[:, sl])

            # graduated first loads so phase A can start early
            for kk, n in ((0, 1), (1, 1), (2, 2), (4, 4)):
                load_batch(kk, n)

            for s in range(NSEG):
                k0, k1 = s * SEG, (s + 1) * SEG
                pt = ptp.tile([SEG, C], F32)
                # ---- phase A (the last tile's column sum is unused) ---
                ka_end = k1 - 1 if s == NSEG - 1 else k1
                for k in range(k0, ka_end):
                    xs = xr[:, k * C:(k + 1) * C]
                    for h in range(NCH):
                        sl = slice(h * CH, (h + 1) * CH)
                        nc.tensor.matmul(
                            pt[:, sl],
                            e_s[:, k * SEG:(k + 1) * SEG],
                            xs[:, sl],
                            start=(k == k0),
                            stop=(k == ka_end - 1),
                        )
                # prefetch next segment's input
                if s + 1 < NSEG:
                    for kk in range((s + 1) * SEG, (s + 2) * SEG, DB):
                        load_batch(kk, DB)
                # copy pt into TOT strip s (bf16, chunked for latency)
                for h in range(NCH):
                    sl = slice(h * CH, (h + 1) * CH)
                    nc.vector.tensor_copy(
                        tot[32 * s:32 * s + SEG, sl], pt[:, sl])

                # ---- phase B -----------------------------------------
                stage = [None] * SEG

                def tril_mm(k):
                    xs = xr[:, k * C:(k + 1) * C]
                    ps = psp.tile([P, C], F32)
                    stage[k - k0] = ps
                    for h in range(NCH):
                        sl = slice(h * CH, (h + 1) * CH)
                        nc.tensor.matmul(
                            ps[:, sl], tril_s[:], xs[:, sl],
                            start=True, stop=(k == 0),
                        )

                def carry_mm(k):
                    K = 32 * s + (k - k0)    # contraction rows of TOT
                    if K == 0:
                        return
                    ps = stage[k - k0]
                    for h in range(NCH):
                        sl = slice(h * CH, (h + 1) * CH)
                        nc.tensor.matmul(
                            ps[:, sl], g_s[0:K, k * P:(k + 1) * P],
                            tot[0:K, sl],
                            start=False, stop=True,
                        )

                def evict(k):
                    ps = stage[k - k0]
                    ob = k // DB
                    o = ostage[ob % len(ostage)]
                    osl = o[:, (k % DB) * C:((k % DB) + 1) * C]
                    scale = r_s[:, k:k + 1]
                    nc.vector.tensor_scalar_mul(
                        osl[:, 0:DVE_COLS], ps[:, 0:DVE_COLS], scale)
                    nc.scalar.activation(
                        osl[:, DVE_COLS:C], ps[:, DVE_COLS:C],
                        mybir.ActivationFunctionType.Copy, scale=scale,
                    )
                    # stores: 1 MiB batches; the final batch ships as
                    # single tiles so the tail drains fast
                    last = (s == NSEG - 1) and (k >= NT - DB)
                    if last:
                        h0 = k % DB
                        sl = slice(k * C, (k + 1) * C)
                        nc.gpsimd.dma_start(
                            out_d[:, sl], o[:, h0 * C:(h0 + 1) * C])
                    elif k % DB == DB - 1:
                        sl = slice((k - DB + 1) * C, (k + 1) * C)
                        nc.gpsimd.dma_start(out_d[:, sl], o[:])

                for k in range(k0, k1):
                    tril_mm(k)
                    carry_mm(k)
                    evict(k)

    nc.compile()
    return nc


def _run(x, trace=False):
    assert x.shape == (B, T, C)
    # partition-major bf16 layout: [B, P, NT*C]
    xb = (
        np.ascontiguousarray(x)
        .astype(ml_dtypes.bfloat16)
        .reshape(B, NT, P, C)
        .transpose(0, 2, 1, 3)
        .reshape(B, P, NT * C)
        .copy()
    )
    if "nc" not in _cache:
        _cache["nc"] = _build()
        _cache["consts"] = _consts()
    nc = _cache["nc"]
    wconst, gb, recip = _cache["consts"]
    in_maps = [
        {"x": xb[b], "wconst": wconst, "gb": gb, "recip": recip}
        for b in range(B)
    ]
    res = run_bass_kernel_spmd(nc, in_maps, core_ids=list(range(B)), trace=trace)
    out = np.stack([
        res.results[b]["out"]
        .astype(np.float32)
        .reshape(P, NT, C)
        .transpose(1, 0, 2)
        .reshape(T, C)
        for b in range(B)
    ])
    return out, res


def kernel(x):
    out, _ = _run(x, trace=False)
    return out
